# revision 7
# baseline (speedup 1.0000x reference)
# Distributed Bass kernel: causal multi-head attention block on 8 TRN2 NeuronCores.
#
# Problem (hardcoded): x [2, 4096, 768] f32, 12 heads x 64 dim, causal attention,
#   out = softmax(mask(q k^T / 8)) v  projected by Wo, all nn.Linear with bias.
#
# Sharding: core c -> batch b = c // 4, head-group hg = c % 4 (3 heads each).
#   Per core: QKV for its 3 heads over the full sequence (tensor parallel on
#   heads), flash-style causal attention, then 8 chunked AllGathers of preout^T
#   (bf16, [192, 512] per rank -> [768, 512]) within each 4-core batch group --
#   pipelined behind attention -- then an output projection sharded over dout
#   (each core computes its own 192 output columns for the full sequence,
#   written transposed [192, 4096] and flipped on the host).
#
# v2 changes vs the original baseline:
#   - host sends x and all weights PRE-TRANSPOSED and PRE-CAST to bf16
#     (xT [768, 4096], W^T [768, m]); kills all on-chip PE transposes,
#     f32->bf16 casts, and the f32 staging DMA (25 MB -> 6.3 MB for x).
#   - variant "pad": q/k tiles span 128 partitions with the upper 64 zeroed,
#     so the qk^T matmul runs contract-128 in (128,128) tile mode like every
#     other matmul in the kernel -- no PE tiling-mode switches at all.
#   - variant "tile": qk^T runs 2x row-tiled (tile_position (0,0)/(64,0)),
#     even sj-chunks' k on partitions 0-63, odd on 64-127, q duplicated in
#     both halves; the two 512-col matmuls of a chunk-pair run concurrently.

import os

import numpy as np

B = 2
S = 4096
D = 768
HD = 64
NH = 12
NCORES = 8
HL = 3            # heads per core
DL = HL * HD      # 192: local q/k/v dims per core
SUP = 512         # si superchunk
NSUP = S // SUP   # 8
NKC = S // 128    # 32 sj chunks
NDC = D // 128    # 6 contraction chunks
GROUPS = [[0, 1, 2, 3], [4, 5, 6, 7]]

VARIANT = os.environ.get("KVAR", "pad")  # "pad" or "tile"

_CACHE = {}


def _build_nc(variant):
    import concourse.mybir as mybir
    from concourse import bacc
    from concourse.tile import TileContext

    f32 = mybir.dt.float32
    bf16 = mybir.dt.bfloat16
    EXP = mybir.ActivationFunctionType.Exp

    nc = bacc.Bacc(num_devices=NCORES)

    xT_p = nc.declare_dram_parameter("xT", [D, S], bf16, isOutput=False)
    wqk_p = nc.declare_dram_parameter("wqk", [D, 2 * DL], bf16, isOutput=False)
    bqk_p = nc.declare_dram_parameter("bqk", [2 * DL, 1], f32, isOutput=False)
    wv_p = nc.declare_dram_parameter("wv", [D, DL], bf16, isOutput=False)
    bv_p = nc.declare_dram_parameter("bv", [DL, 1], f32, isOutput=False)
    wo_p = nc.declare_dram_parameter("wo", [D, DL], bf16, isOutput=False)
    bo_p = nc.declare_dram_parameter("bo", [DL, 1], f32, isOutput=False)
    out_p = nc.declare_dram_parameter("out", [DL, S], f32, isOutput=True)

    NCHUNK = 16
    CW = S // NCHUNK  # 256 columns per AllGather chunk
    cins = [nc.dram_tensor(f"cc_in{c}", [DL, CW], bf16) for c in range(NCHUNK)]
    couts = [nc.dram_tensor(f"cc_out{c}", [D, CW], bf16) for c in range(NCHUNK)]

    tiled = variant == "tile"

    with TileContext(nc) as tc:
        with (
            tc.tile_pool(name="const", bufs=1) as cpool,
            tc.tile_pool(name="at", bufs=3) as atpool,
            tc.tile_pool(name="ps", bufs=2) as pspool,
            tc.tile_pool(name="bc", bufs=2) as bcpool,
            tc.tile_pool(name="ot", bufs=2) as otpool,
            tc.tile_pool(name="mm", bufs=2, space="PSUM") as mmpsum,
            tc.tile_pool(name="lg", bufs=2, space="PSUM") as lgpsum,
            tc.tile_pool(name="po", bufs=2, space="PSUM") as popsum,
        ):
            # ---------------- constants / weights ----------------
            # DMA order matters: the first projection matmul needs wqk + the
            # first x^T superchunk, so those go to the queue first.
            wqk_sb = cpool.tile([128, NDC, 2 * DL], bf16, name="wqk_sb")
            nc.sync.dma_start(
                out=wqk_sb[:, :, :],
                in_=wqk_p[:, :].rearrange("(c p) m -> p c m", p=128),
            )
            # x^T in bf16, DMA'd straight from DRAM (no transposes, no casts).
            xT = cpool.tile([128, NDC, S], bf16, name="xT")  # 48KB/partition
            nc.sync.dma_start(
                out=xT[:, :, 0:SUP],
                in_=xT_p[:, :].rearrange("(c p) s -> p c s", p=128)[:, :, 0:SUP],
            )
            wv_sb = cpool.tile([128, NDC, DL], bf16, name="wv_sb")
            nc.sync.dma_start(
                out=wv_sb[:, :, :],
                in_=wv_p[:, :].rearrange("(c p) m -> p c m", p=128),
            )
            bqk_sb = cpool.tile([128, 2 * DL // 128, 1], f32, name="bqk_sb")
            nc.sync.dma_start(
                out=bqk_sb[:, :, :], in_=bqk_p[:, :].rearrange("(c p) o -> p c o", p=128)
            )
            wo_sb = cpool.tile([128, NDC, DL], bf16, name="wo_sb")
            nc.sync.dma_start(
                out=wo_sb[:, :, :],
                in_=wo_p[:, :].rearrange("(c p) m -> p c m", p=128),
            )
            bv_sb = cpool.tile([64, HL, 1], f32, name="bv_sb")
            nc.sync.dma_start(
                out=bv_sb[:, :, :], in_=bv_p[:, :].rearrange("(h p) o -> p h o", p=64)
            )
            bo0_sb = cpool.tile([128, 1], f32, name="bo0_sb")
            nc.sync.dma_start(out=bo0_sb[:, :], in_=bo_p[0:128, :])
            bo1_sb = cpool.tile([64, 1], f32, name="bo1_sb")
            nc.sync.dma_start(out=bo1_sb[:, :], in_=bo_p[128:DL, :])

            # multiplicative causal masks for the 4 diagonal sj-chunk offsets:
            # masks[p, k, f] = 1.0 if (f - p - 128k) >= 0 else 0.0
            masks = cpool.tile([128, 4, SUP], bf16, name="masks")
            nc.gpsimd.memset(masks[:, :, :], 1.0)
            for k in range(4):
                nc.gpsimd.affine_select(
                    out=masks[:, k, :],
                    in_=masks[:, k, :],
                    compare_op=mybir.AluOpType.is_ge,
                    fill=0.0,
                    base=-128 * k,
                    pattern=[[1, SUP]],
                    channel_multiplier=-1,
                )

            if tiled:
                # q duplicated in both partition halves; k split by sj-chunk
                # parity: even chunks at partitions 0-63 (PE tile T0), odd at
                # 64-127 (T8). kT2[64h + p, hd, j2, i] = k chunk 2*j2(+1).
                qT = cpool.tile([128, HL, S], bf16, name="qT")
                kT = cpool.tile([128, HL, NKC // 2, 128], bf16, name="kT")
            else:
                # contract-128 zero-padded: real data at partitions 0-63,
                # zeros at 64-127 (k side; q upper half zeroed too so no
                # NaN garbage enters the array).
                qT = cpool.tile([128, HL, S], bf16, name="qT")
                kT = cpool.tile([128, HL, S], bf16, name="kT")
                nc.gpsimd.memset(kT[64:128, :, :], 0.0)
                nc.gpsimd.memset(qT[64:128, :, :], 0.0)

            v65 = cpool.tile([128, NKC, HL * (HD + 1)], bf16, name="v65")
            poT = cpool.tile([64, HL, S], bf16, name="poT")  # preout^T, per head

            # ones column of v' (col 64 of each head's 65-wide block)
            nc.vector.memset(v65[:, :, :], 1.0)

            # ---------------- phase 1: x^T DMA + qk/v projections ----------------
            for t in range(NSUP):
                if t > 0:
                    nc.sync.dma_start(
                        out=xT[:, :, t * SUP : (t + 1) * SUP],
                        in_=xT_p[:, :].rearrange("(c p) s -> p c s", p=128)[
                            :, :, t * SUP : (t + 1) * SUP
                        ],
                    )

                # q/k projection for this superchunk: out [m, s]
                for mc in range(2 * DL // 128):
                    ps = mmpsum.tile([128, 512], f32, name="ps", tag="mm")
                    for dc in range(NDC):
                        nc.tensor.matmul(
                            ps[:, :],
                            lhsT=wqk_sb[:, dc, mc * 128 : (mc + 1) * 128],
                            rhs=xT[:, dc, t * SUP : (t + 1) * SUP],
                            start=(dc == 0),
                            stop=(dc == NDC - 1),
                        )
                    for half in (0, 1):
                        g = mc * 128 + half * 64  # global row in [q(192); k(192)]
                        src = ps[half * 64 : half * 64 + 64, :]
                        bias = bqk_sb[half * 64 : half * 64 + 64, mc, :]
                        if g < DL:
                            h = g // 64
                            nc.vector.tensor_scalar_add(
                                qT[0:64, h, t * SUP : (t + 1) * SUP], src, bias
                            )
                        else:
                            h = (g - DL) // 64
                            if tiled:
                                # split even/odd sj-chunks into partition
                                # halves; chunk 4t+{0,2} -> even slots
                                # {2t, 2t+1}, chunk 4t+{1,3} -> odd slots.
                                s4 = src.rearrange("p (a b c) -> p a b c", a=2, b=2)
                                nc.vector.tensor_scalar_add(
                                    kT[0:64, h, 2 * t : 2 * t + 2, :],
                                    s4[:, :, 0, :],
                                    bias,
                                )
                                nc.vector.tensor_scalar_add(
                                    kT[64:128, h, 2 * t : 2 * t + 2, :],
                                    s4[:, :, 1, :],
                                    bias,
                                )
                            else:
                                nc.vector.tensor_scalar_add(
                                    kT[0:64, h, t * SUP : (t + 1) * SUP], src, bias
                                )
                if tiled:
                    # duplicate this superchunk's q rows into the upper
                    # partition half (SBUF->SBUF, feeds the T8 row-tile).
                    nc.vector.tensor_copy(
                        qT[64:128, :, t * SUP : (t + 1) * SUP],
                        qT[0:64, :, t * SUP : (t + 1) * SUP],
                    )

                # v for this superchunk: out [s, m] (bias deferred to post-softmax)
                for sub in range(4):
                    j = t * 4 + sub
                    pv = mmpsum.tile([128, 512], f32, name="pv", tag="mm")
                    for dc in range(NDC):
                        nc.tensor.matmul(
                            pv[:, 0:DL],
                            lhsT=xT[:, dc, j * 128 : (j + 1) * 128],
                            rhs=wv_sb[:, dc, :],
                            start=(dc == 0),
                            stop=(dc == NDC - 1),
                        )
                    nc.vector.tensor_copy(
                        v65[:, j, :].rearrange("p (h w) -> p h w", h=HL)[:, :, 0:HD],
                        pv[:, 0:DL].rearrange("p (h w) -> p h w", h=HL),
                    )

            # ---------------- phase 2: flash attention (logits transposed) ----------------
            # Software-pipelined emission: the PE executes its queue in order,
            # so qk of pair k+1 is emitted BEFORE a@v of pair k -- the qk
            # matmuls then run while the scalar engine computes exp(pair k),
            # instead of the PE idling behind exp each pair.
            tasks = [
                (t, h, pr)
                for t in range(NSUP)
                for h in range(HL)
                for pr in range(2 * t + 2)
            ]

            def emit_qk(task):
                t, h, pr = task
                si0 = t * SUP
                off = 256 if pr == 2 * t + 1 else 0
                lg = lgpsum.tile([128, 2, 512], f32, name="lg", tag="lg")
                aT = atpool.tile([128, 2, 512], bf16, name="aT", tag="at")
                for half in (0, 1):
                    j = 2 * pr + half
                    if tiled:
                        p0 = 64 * half
                        nc.tensor.matmul(
                            lg[:, half, off:],
                            lhsT=kT[p0 : p0 + 64, h, pr, :],
                            rhs=qT[p0 : p0 + 64, h, si0 + off : si0 + SUP],
                            start=True,
                            stop=True,
                            tile_position=(p0, 0),
                        )
                    else:
                        sj0 = 128 * j
                        nc.tensor.matmul(
                            lg[:, half, off:],
                            lhsT=kT[:, h, sj0 : sj0 + 128],
                            rhs=qT[:, h, si0 + off : si0 + SUP],
                            start=True,
                            stop=True,
                        )
                # exp of both halves in one ACT instruction (scale = 1/8)
                nc.scalar.activation(aT[:, :, off:], lg[:, :, off:], EXP, scale=0.125)
                for half in (0, 1):
                    j = 2 * pr + half
                    krel = j - 4 * t
                    if krel >= 0:  # diagonal chunk: multiplicative causal mask
                        nc.vector.tensor_mul(
                            aT[:, half, off:],
                            aT[:, half, off:],
                            masks[:, krel, off:],
                        )
                return aT, off

            po_tiles = {}

            def emit_av(task, aT, off):
                t, h, pr = task
                n_j = 4 * t + 4
                if pr == 0:
                    po_tiles[(t, h)] = popsum.tile([65, 512], f32, name="po", tag="po")
                po = po_tiles[(t, h)]
                for half in (0, 1):
                    j = 2 * pr + half
                    nc.tensor.matmul(
                        po[:, off:],
                        lhsT=v65[:, j, :].rearrange("p (hh w) -> p hh w", hh=HL)[
                            :, h, :
                        ],
                        rhs=aT[:, half, off:],
                        start=(j == 0),
                        stop=(j == n_j - 1),
                    )
                if pr < 2 * t + 1:
                    return
                # last pair of (t, h): normalize by softmax denominator
                # (row 64) + deferred v bias.
                si0 = t * SUP
                rc = pspool.tile([1, 512], f32, name="rc", tag="rc")
                nc.vector.tensor_copy(rc[:, :], po[64:65, :])
                bcs = bcpool.tile([64, 512], f32, name="bcs", tag="bc")
                nc.gpsimd.partition_broadcast(bcs[:, :], rc[:, :], channels=64)
                nc.vector.reciprocal_approx_fast(out=bcs[:, :], in_=bcs[:, :])
                nc.vector.tensor_mul(poT[:, h, si0 : si0 + SUP], po[0:64, :], bcs[:, :])
                nc.vector.tensor_scalar_add(
                    poT[:, h, si0 : si0 + SUP],
                    poT[:, h, si0 : si0 + SUP],
                    bv_sb[:, h, :],
                )
                del po_tiles[(t, h)]
                if h < HL - 1:
                    return
                # ---------------- phase 3: chunked AllGather (2 per superchunk) ----------------
                for c in (2 * t, 2 * t + 1):
                    for hh in range(HL):
                        nc.sync.dma_start(
                            out=cins[c][HD * hh : HD * (hh + 1), :],
                            in_=poT[:, hh, c * CW : (c + 1) * CW],
                        )
                    nc.gpsimd.collective_compute(
                        "AllGather",
                        mybir.AluOpType.bypass,
                        replica_groups=GROUPS,
                        ins=[cins[c][:, :]],
                        outs=[couts[c][:, :]],
                    )

            prev = None
            for task in tasks:
                cur = (task, *emit_qk(task))
                if prev is not None:
                    emit_av(*prev)
                prev = cur
            emit_av(*prev)

            # ---------------- phase 4: output projection (dout-sharded) ----------------
            # couts chunks are 256 cols; pair them back into 512-col strips so
            # the matmuls stay at N=512.
            with tc.tile_pool(name="ccp", bufs=1) as ccpool:
                for p in range(NCHUNK // 2):
                    strips = []
                    for dc in range(NDC):
                        strip = ccpool.tile(
                            [128, 2, CW], bf16, name=f"ccs{p}_{dc}", tag=f"ccs{dc}", bufs=2
                        )
                        for half in (0, 1):
                            c = 2 * p + half
                            nc.sync.dma_start(
                                out=strip[:, half, :],
                                in_=couts[c][dc * 128 : (dc + 1) * 128, :],
                            )
                        strips.append(strip)
                    for oc, M0, bo_sb in ((0, 128, bo0_sb), (1, 64, bo1_sb)):
                        pso = mmpsum.tile([128, 512], f32, name="pso", tag="mm")
                        for dc in range(NDC):
                            nc.tensor.matmul(
                                pso[0:M0, :],
                                lhsT=wo_sb[:, dc, oc * 128 : oc * 128 + M0],
                                rhs=strips[dc][:, :, :],
                                start=(dc == 0),
                                stop=(dc == NDC - 1),
                            )
                        ot = otpool.tile([128, 512], f32, name="ot", tag="ot")
                        nc.vector.tensor_scalar_add(
                            ot[0:M0, :], pso[0:M0, :], bo_sb[:, :]
                        )
                        nc.sync.dma_start(
                            out=out_p[
                                oc * 128 : oc * 128 + M0, p * SUP : (p + 1) * SUP
                            ],
                            in_=ot[0:M0, :],
                        )

    nc.finalize()
    return nc


def _get_nc():
    if "nc" not in _CACHE:
        _CACHE["nc"] = _build_nc(VARIANT)
    return _CACHE["nc"]


def _make_in_maps(x, Wq_w, Wq_b, Wk_w, Wk_b, Wv_w, Wv_b, Wo_w, Wo_b):
    import ml_dtypes

    bf = ml_dtypes.bfloat16
    f = np.float32
    in_maps = []
    for c in range(NCORES):
        b, hg = divmod(c, 4)
        r = slice(hg * DL, (hg + 1) * DL)
        in_maps.append(
            {
                "xT": np.ascontiguousarray(x[b].T.astype(bf)),
                "wqk": np.ascontiguousarray(
                    np.concatenate([Wq_w[r], Wk_w[r]], axis=0).T.astype(bf)
                ),
                "bqk": np.ascontiguousarray(
                    np.concatenate([Wq_b[r], Wk_b[r]])[:, None], dtype=f
                ),
                "wv": np.ascontiguousarray(Wv_w[r].T.astype(bf)),
                "bv": np.ascontiguousarray(Wv_b[r][:, None], dtype=f),
                "wo": np.ascontiguousarray(Wo_w[r].T.astype(bf)),
                "bo": np.ascontiguousarray(Wo_b[r][:, None], dtype=f),
            }
        )
    return in_maps


def run_on_hw(in_maps, trace=False):
    from concourse.bass_utils import run_bass_kernel_spmd

    nc = _get_nc()
    return run_bass_kernel_spmd(nc, in_maps, core_ids=list(range(NCORES)), trace=trace)


def kernel(x, Wq_w, Wq_b, Wk_w, Wk_b, Wv_w, Wv_b, Wo_w, Wo_b):
    in_maps = _make_in_maps(
        np.asarray(x, dtype=np.float32),
        *[
            np.asarray(a, dtype=np.float32)
            for a in (Wq_w, Wq_b, Wk_w, Wk_b, Wv_w, Wv_b, Wo_w, Wo_b)
        ],
    )
    res = run_on_hw(in_maps, trace=False)
    out = np.empty((B, S, D), dtype=np.float32)
    for c in range(NCORES):
        b, hg = divmod(c, 4)
        out[b, :, hg * DL : (hg + 1) * DL] = res.results[c]["out"].T
    return out


# revision 13
# speedup vs baseline: 1.1109x; 1.1109x over previous
# Distributed Bass kernel: causal multi-head attention block on 8 TRN2 NeuronCores.
#
# Problem (hardcoded): x [2, 4096, 768] f32, 12 heads x 64 dim, causal attention,
#   out = softmax(mask(q k^T / 8)) v  projected by Wo, all nn.Linear with bias.
#
# Sharding: core c -> batch b = c // 4, head-group hg = c % 4 (3 heads each).
#   Per core: QKV for its 3 heads over the full sequence (tensor parallel on
#   heads), flash-style causal attention, then 8 chunked AllGathers of preout^T
#   (bf16, [192, 512] per rank -> [768, 512]) within each 4-core batch group --
#   pipelined behind attention -- then an output projection sharded over dout
#   (each core computes its own 192 output columns for the full sequence,
#   written transposed [192, 4096] and flipped on the host).
#
# v2 changes vs the original baseline:
#   - host sends x and all weights PRE-TRANSPOSED and PRE-CAST to bf16
#     (xT [768, 4096], W^T [768, m]); kills all on-chip PE transposes,
#     f32->bf16 casts, and the f32 staging DMA (25 MB -> 6.3 MB for x).
#   - variant "pad": q/k tiles span 128 partitions with the upper 64 zeroed,
#     so the qk^T matmul runs contract-128 in (128,128) tile mode like every
#     other matmul in the kernel -- no PE tiling-mode switches at all.
#   - variant "tile": qk^T runs 2x row-tiled (tile_position (0,0)/(64,0)),
#     even sj-chunks' k on partitions 0-63, odd on 64-127, q duplicated in
#     both halves; the two 512-col matmuls of a chunk-pair run concurrently.

import os

import numpy as np

B = 2
S = 4096
D = 768
HD = 64
NH = 12
NCORES = 8
HL = 3            # heads per core
DL = HL * HD      # 192: local q/k/v dims per core
SUP = 512         # si superchunk
NSUP = S // SUP   # 8
NKC = S // 128    # 32 sj chunks
NDC = D // 128    # 6 contraction chunks
GROUPS = [[0, 1, 2, 3], [4, 5, 6, 7]]

VARIANT = os.environ.get("KVAR", "pad")  # "pad" or "tile"

_CACHE = {}


def _build_nc(variant):
    import concourse.mybir as mybir
    from concourse import bacc
    from concourse.tile import TileContext

    f32 = mybir.dt.float32
    bf16 = mybir.dt.bfloat16
    EXP = mybir.ActivationFunctionType.Exp

    nc = bacc.Bacc(num_devices=NCORES)

    xT_p = nc.declare_dram_parameter("xT", [D, S], bf16, isOutput=False)
    wqk_p = nc.declare_dram_parameter("wqk", [D, 2 * DL], bf16, isOutput=False)
    bqk_p = nc.declare_dram_parameter("bqk", [2 * DL, 1], f32, isOutput=False)
    wv_p = nc.declare_dram_parameter("wv", [D, DL], bf16, isOutput=False)
    bv_p = nc.declare_dram_parameter("bv", [DL, 1], f32, isOutput=False)
    wo_p = nc.declare_dram_parameter("wo", [D, DL], bf16, isOutput=False)
    bo_p = nc.declare_dram_parameter("bo", [DL, 1], f32, isOutput=False)
    out_p = nc.declare_dram_parameter("out", [DL, S], f32, isOutput=True)

    NCHUNK = 8
    CW = S // NCHUNK  # 512 columns per AllGather chunk
    cins = [nc.dram_tensor(f"cc_in{c}", [DL, CW], bf16) for c in range(NCHUNK)]
    couts = [nc.dram_tensor(f"cc_out{c}", [D, CW], bf16) for c in range(NCHUNK)]

    tiled = variant == "tile"

    with TileContext(nc) as tc:
        with (
            tc.tile_pool(name="const", bufs=1) as cpool,
            tc.tile_pool(name="at", bufs=3) as atpool,
            tc.tile_pool(name="ps", bufs=2) as pspool,
            tc.tile_pool(name="ot", bufs=2) as otpool,
            tc.tile_pool(name="mm", bufs=2, space="PSUM") as mmpsum,
            tc.tile_pool(name="lg", bufs=2, space="PSUM") as lgpsum,
            tc.tile_pool(name="po", bufs=2, space="PSUM") as popsum,
        ):
            # ---------------- constants / weights ----------------
            # DMA order matters: the first projection matmul needs wqk + the
            # first x^T superchunk, so those go to the queue first.
            wqk_sb = cpool.tile([128, NDC, 2 * DL], bf16, name="wqk_sb")
            nc.sync.dma_start(
                out=wqk_sb[:, :, :],
                in_=wqk_p[:, :].rearrange("(c p) m -> p c m", p=128),
            )
            # x^T in bf16, DMA'd straight from DRAM (no transposes, no casts).
            xT = cpool.tile([128, NDC, S], bf16, name="xT")  # 48KB/partition
            nc.sync.dma_start(
                out=xT[:, :, 0:SUP],
                in_=xT_p[:, :].rearrange("(c p) s -> p c s", p=128)[:, :, 0:SUP],
            )
            wv_sb = cpool.tile([128, NDC, DL], bf16, name="wv_sb")
            nc.sync.dma_start(
                out=wv_sb[:, :, :],
                in_=wv_p[:, :].rearrange("(c p) m -> p c m", p=128),
            )
            bqk_sb = cpool.tile([128, 2 * DL // 128, 1], f32, name="bqk_sb")
            nc.sync.dma_start(
                out=bqk_sb[:, :, :], in_=bqk_p[:, :].rearrange("(c p) o -> p c o", p=128)
            )
            wo_sb = cpool.tile([128, NDC, DL], bf16, name="wo_sb")
            nc.sync.dma_start(
                out=wo_sb[:, :, :],
                in_=wo_p[:, :].rearrange("(c p) m -> p c m", p=128),
            )
            bv_sb = cpool.tile([64, HL, 1], f32, name="bv_sb")
            nc.sync.dma_start(
                out=bv_sb[:, :, :], in_=bv_p[:, :].rearrange("(h p) o -> p h o", p=64)
            )
            bo0_sb = cpool.tile([128, 1], f32, name="bo0_sb")
            nc.sync.dma_start(out=bo0_sb[:, :], in_=bo_p[0:128, :])
            bo1_sb = cpool.tile([64, 1], f32, name="bo1_sb")
            nc.sync.dma_start(out=bo1_sb[:, :], in_=bo_p[128:DL, :])

            # multiplicative causal masks for the 4 diagonal sj-chunk offsets:
            # masks[p, k, f] = 1.0 if (f - p - 128k) >= 0 else 0.0
            masks = cpool.tile([128, 4, SUP], bf16, name="masks")
            nc.gpsimd.memset(masks[:, :, :], 1.0)
            for k in range(4):
                nc.gpsimd.affine_select(
                    out=masks[:, k, :],
                    in_=masks[:, k, :],
                    compare_op=mybir.AluOpType.is_ge,
                    fill=0.0,
                    base=-128 * k,
                    pattern=[[1, SUP]],
                    channel_multiplier=-1,
                )

            if tiled:
                # q duplicated in both partition halves; k split by sj-chunk
                # parity: even chunks at partitions 0-63 (PE tile T0), odd at
                # 64-127 (T8). kT2[64h + p, hd, j2, i] = k chunk 2*j2(+1).
                qT = cpool.tile([128, HL, S], bf16, name="qT")
                kT = cpool.tile([128, HL, NKC // 2, 128], bf16, name="kT")
            else:
                # contract-128 zero-padded: real data at partitions 0-63,
                # zeros at 64-127 (k side; q upper half zeroed too so no
                # NaN garbage enters the array).
                qT = cpool.tile([128, HL, S], bf16, name="qT")
                kT = cpool.tile([128, HL, S], bf16, name="kT")
                nc.gpsimd.memset(kT[64:128, :, :], 0.0)
                nc.gpsimd.memset(qT[64:128, :, :], 0.0)

            # v augmented to 128 cols per head: cols 0-63 the values, cols
            # 64-127 all ones -- the a@v matmul then emits the softmax
            # denominator REPLICATED on po partitions 64-127, so no
            # partition-broadcast is needed for the normalize (and gpsimd
            # carries nothing but the AllGathers).
            v128 = cpool.tile([128, NKC, HL, 128], bf16, name="v128")
            poT = cpool.tile([64, HL, S], bf16, name="poT")  # preout^T, per head
            nc.vector.memset(v128[:, :, :, :], 1.0)

            # ---------------- phase 1: x^T DMA + qk/v projections ----------------
            for t in range(NSUP):
                if t > 0:
                    nc.sync.dma_start(
                        out=xT[:, :, t * SUP : (t + 1) * SUP],
                        in_=xT_p[:, :].rearrange("(c p) s -> p c s", p=128)[
                            :, :, t * SUP : (t + 1) * SUP
                        ],
                    )

                # q/k projection for this superchunk: out [m, s]
                for mc in range(2 * DL // 128):
                    ps = mmpsum.tile([128, 512], f32, name="ps", tag="mm")
                    for dc in range(NDC):
                        nc.tensor.matmul(
                            ps[:, :],
                            lhsT=wqk_sb[:, dc, mc * 128 : (mc + 1) * 128],
                            rhs=xT[:, dc, t * SUP : (t + 1) * SUP],
                            start=(dc == 0),
                            stop=(dc == NDC - 1),
                        )
                    for half in (0, 1):
                        g = mc * 128 + half * 64  # global row in [q(192); k(192)]
                        src = ps[half * 64 : half * 64 + 64, :]
                        bias = bqk_sb[half * 64 : half * 64 + 64, mc, :]
                        if g < DL:
                            h = g // 64
                            nc.vector.tensor_scalar_add(
                                qT[0:64, h, t * SUP : (t + 1) * SUP], src, bias
                            )
                        else:
                            h = (g - DL) // 64
                            if tiled:
                                # split even/odd sj-chunks into partition
                                # halves; chunk 4t+{0,2} -> even slots
                                # {2t, 2t+1}, chunk 4t+{1,3} -> odd slots.
                                s4 = src.rearrange("p (a b c) -> p a b c", a=2, b=2)
                                nc.vector.tensor_scalar_add(
                                    kT[0:64, h, 2 * t : 2 * t + 2, :],
                                    s4[:, :, 0, :],
                                    bias,
                                )
                                nc.vector.tensor_scalar_add(
                                    kT[64:128, h, 2 * t : 2 * t + 2, :],
                                    s4[:, :, 1, :],
                                    bias,
                                )
                            else:
                                nc.vector.tensor_scalar_add(
                                    kT[0:64, h, t * SUP : (t + 1) * SUP], src, bias
                                )
                if tiled:
                    # duplicate this superchunk's q rows into the upper
                    # partition half (SBUF->SBUF, feeds the T8 row-tile).
                    nc.vector.tensor_copy(
                        qT[64:128, :, t * SUP : (t + 1) * SUP],
                        qT[0:64, :, t * SUP : (t + 1) * SUP],
                    )

                # v for this superchunk: out [s, m] (bias deferred to post-softmax)
                for sub in range(4):
                    j = t * 4 + sub
                    pv = mmpsum.tile([128, 512], f32, name="pv", tag="mm")
                    for dc in range(NDC):
                        nc.tensor.matmul(
                            pv[:, 0:DL],
                            lhsT=xT[:, dc, j * 128 : (j + 1) * 128],
                            rhs=wv_sb[:, dc, :],
                            start=(dc == 0),
                            stop=(dc == NDC - 1),
                        )
                    nc.vector.tensor_copy(
                        v128[:, j, :, 0:HD],
                        pv[:, 0:DL].rearrange("p (h w) -> p h w", h=HL),
                    )

            # ---------------- phase 2: flash attention (logits transposed) ----------------
            # Software-pipelined emission: the PE executes its queue in order,
            # so qk of pair k+1 is emitted BEFORE a@v of pair k -- the qk
            # matmuls then run while the scalar engine computes exp(pair k),
            # instead of the PE idling behind exp each pair.
            tasks = [
                (t, h, pr)
                for t in range(NSUP)
                for h in range(HL)
                for pr in range(2 * t + 2)
            ]

            def emit_qk(task):
                t, h, pr = task
                si0 = t * SUP
                off = 256 if pr == 2 * t + 1 else 0
                lg = lgpsum.tile([128, 2, 512], f32, name="lg", tag="lg")
                aT = atpool.tile([128, 2, 512], bf16, name="aT", tag="at")
                for half in (0, 1):
                    j = 2 * pr + half
                    if tiled:
                        p0 = 64 * half
                        nc.tensor.matmul(
                            lg[:, half, off:],
                            lhsT=kT[p0 : p0 + 64, h, pr, :],
                            rhs=qT[p0 : p0 + 64, h, si0 + off : si0 + SUP],
                            start=True,
                            stop=True,
                            tile_position=(p0, 0),
                        )
                    else:
                        sj0 = 128 * j
                        nc.tensor.matmul(
                            lg[:, half, off:],
                            lhsT=kT[:, h, sj0 : sj0 + 128],
                            rhs=qT[:, h, si0 + off : si0 + SUP],
                            start=True,
                            stop=True,
                        )
                # exp of both halves in one ACT instruction (scale = 1/8)
                nc.scalar.activation(aT[:, :, off:], lg[:, :, off:], EXP, scale=0.125)
                for half in (0, 1):
                    j = 2 * pr + half
                    krel = j - 4 * t
                    if krel >= 0:  # diagonal chunk: multiplicative causal mask
                        nc.vector.tensor_mul(
                            aT[:, half, off:],
                            aT[:, half, off:],
                            masks[:, krel, off:],
                        )
                return aT, off

            po_tiles = {}

            def emit_av(task, aT, off):
                t, h, pr = task
                n_j = 4 * t + 4
                if pr == 0:
                    po_tiles[(t, h)] = popsum.tile([128, 512], f32, name="po", tag="po")
                po = po_tiles[(t, h)]
                for half in (0, 1):
                    j = 2 * pr + half
                    nc.tensor.matmul(
                        po[:, off:],
                        lhsT=v128[:, j, h, :],
                        rhs=aT[:, half, off:],
                        start=(j == 0),
                        stop=(j == n_j - 1),
                    )
                if pr < 2 * t + 1:
                    return
                # last pair of (t, h): normalize by the replicated softmax
                # denominator (po rows 64-127) + deferred v bias. Pure DVE.
                si0 = t * SUP
                bcs = pspool.tile([64, 512], f32, name="bcs", tag="bc")
                nc.vector.tensor_copy(bcs[:, :], po[64:128, :])
                nc.vector.reciprocal_approx_fast(out=bcs[:, :], in_=bcs[:, :])
                nc.vector.tensor_mul(
                    poT[:, h, si0 : si0 + SUP], po[0:64, :], bcs[:, :]
                )
                nc.vector.tensor_scalar_add(
                    poT[:, h, si0 : si0 + SUP],
                    poT[:, h, si0 : si0 + SUP],
                    bv_sb[:, h, :],
                )
                del po_tiles[(t, h)]
                if h < HL - 1:
                    return
                # ---------------- phase 3: chunked AllGather ----------------
                c = t
                for hh in range(HL):
                    nc.sync.dma_start(
                        out=cins[c][HD * hh : HD * (hh + 1), :],
                        in_=poT[:, hh, c * CW : (c + 1) * CW],
                    )
                nc.gpsimd.collective_compute(
                    "AllGather",
                    mybir.AluOpType.bypass,
                    replica_groups=GROUPS,
                    ins=[cins[c][:, :]],
                    outs=[couts[c][:, :]],
                )

            prev = None
            for task in tasks:
                cur = (task, *emit_qk(task))
                if prev is not None:
                    emit_av(*prev)
                prev = cur
            emit_av(*prev)

            # ---------------- phase 4: output projection (dout-sharded) ----------------
            with tc.tile_pool(name="ccp", bufs=1) as ccpool:
                for c in range(NCHUNK):
                    strips = []
                    for dc in range(NDC):
                        strip = ccpool.tile(
                            [128, CW], bf16, name=f"ccs{c}_{dc}", tag=f"ccs{dc}", bufs=2
                        )
                        nc.sync.dma_start(
                            out=strip[:, :], in_=couts[c][dc * 128 : (dc + 1) * 128, :]
                        )
                        strips.append(strip)
                    for oc, M0, bo_sb in ((0, 128, bo0_sb), (1, 64, bo1_sb)):
                        pso = mmpsum.tile([128, 512], f32, name="pso", tag="mm")
                        for dc in range(NDC):
                            nc.tensor.matmul(
                                pso[0:M0, :],
                                lhsT=wo_sb[:, dc, oc * 128 : oc * 128 + M0],
                                rhs=strips[dc][:, :],
                                start=(dc == 0),
                                stop=(dc == NDC - 1),
                            )
                        ot = otpool.tile([128, 512], f32, name="ot", tag="ot")
                        nc.vector.tensor_scalar_add(
                            ot[0:M0, :], pso[0:M0, :], bo_sb[:, :]
                        )
                        nc.sync.dma_start(
                            out=out_p[
                                oc * 128 : oc * 128 + M0, c * SUP : (c + 1) * SUP
                            ],
                            in_=ot[0:M0, :],
                        )

    nc.finalize()
    return nc


def _get_nc():
    if "nc" not in _CACHE:
        _CACHE["nc"] = _build_nc(VARIANT)
    return _CACHE["nc"]


def _make_in_maps(x, Wq_w, Wq_b, Wk_w, Wk_b, Wv_w, Wv_b, Wo_w, Wo_b):
    import ml_dtypes

    bf = ml_dtypes.bfloat16
    f = np.float32
    in_maps = []
    for c in range(NCORES):
        b, hg = divmod(c, 4)
        r = slice(hg * DL, (hg + 1) * DL)
        in_maps.append(
            {
                "xT": np.ascontiguousarray(x[b].T.astype(bf)),
                "wqk": np.ascontiguousarray(
                    np.concatenate([Wq_w[r], Wk_w[r]], axis=0).T.astype(bf)
                ),
                "bqk": np.ascontiguousarray(
                    np.concatenate([Wq_b[r], Wk_b[r]])[:, None], dtype=f
                ),
                "wv": np.ascontiguousarray(Wv_w[r].T.astype(bf)),
                "bv": np.ascontiguousarray(Wv_b[r][:, None], dtype=f),
                "wo": np.ascontiguousarray(Wo_w[r].T.astype(bf)),
                "bo": np.ascontiguousarray(Wo_b[r][:, None], dtype=f),
            }
        )
    return in_maps


def run_on_hw(in_maps, trace=False):
    from concourse.bass_utils import run_bass_kernel_spmd

    nc = _get_nc()
    return run_bass_kernel_spmd(nc, in_maps, core_ids=list(range(NCORES)), trace=trace)


def kernel(x, Wq_w, Wq_b, Wk_w, Wk_b, Wv_w, Wv_b, Wo_w, Wo_b):
    in_maps = _make_in_maps(
        np.asarray(x, dtype=np.float32),
        *[
            np.asarray(a, dtype=np.float32)
            for a in (Wq_w, Wq_b, Wk_w, Wk_b, Wv_w, Wv_b, Wo_w, Wo_b)
        ],
    )
    res = run_on_hw(in_maps, trace=False)
    out = np.empty((B, S, D), dtype=np.float32)
    for c in range(NCORES):
        b, hg = divmod(c, 4)
        out[b, :, hg * DL : (hg + 1) * DL] = res.results[c]["out"].T
    return out
